# revision 11
# baseline (speedup 1.0000x reference)
"""MiniMax MoE block on 8 trn2 NeuronCores.

Expert-parallel: one expert per core. The tiny router (T x D @ D x E) and the
top-k dispatch run on host as the sharding step; each core runs the SwiGLU
expert MLP over the tokens routed to it (capacity-padded), and the host
scatter-adds the weighted expert outputs back (the combine weight commutes
through the down projection, so it is applied during the combine).

Shapes (hardcoded): B=2, S=1024, D=2048, E=8, F=1024, top_k=2.
"""

import os

import numpy as np

B, S, D, E, F = 2, 1024, 2048, 8, 1024
T = B * S
P = 128
NT = 512  # max token chunk (matmul moving dim; fp32 max is 512)
CQ = 256  # capacity quantum (fp32r needs moving dim >= 256 for full rate)
DK = D // P  # 16 contraction tiles over D
FK = F // P  # 8 contraction tiles over F


def _chunks(C):
    """Split C (multiple of 256) into balanced chunks of 256..512, each a
    multiple of 128, so every fp32r matmul keeps the full-rate moving dim."""
    k = (C + NT - 1) // NT
    base = (C // k) // P * P
    out = [base] * k
    rem = C - base * k
    i = 0
    while rem:
        out[i] += P
        rem -= P
        i += 1
    return out

_COMPILED = {}
LAST_RESULT = None


def _round_fp32r(a):
    """Round fp32 to the PE's fp32r format (11 mantissa bits, RNE)."""
    b = np.ascontiguousarray(a, np.float32).view(np.uint32)
    b = b + np.uint32(0x7FF) + ((b >> np.uint32(12)) & np.uint32(1))
    b &= np.uint32(0xFFFFF000)
    return b.view(np.float32)


def _route(x, gate_w, k):
    """Router logits + top-k combine weights, matching reference numerics.

    Uses jax on CPU with the exact op sequence of the reference so the top-k
    selection is bit-identical; falls back to numpy if jax is unavailable.
    """
    try:
        import jax
        import jax.numpy as jnp

        cpu = jax.devices("cpu")[0]
        with jax.default_device(cpu):
            xl = jnp.asarray(x)
            gw = jnp.asarray(gate_w)
            logits = jnp.einsum("td,ed->te", xl, gw)
            probs = jax.nn.softmax(logits, axis=-1)
            tw, ti = jax.lax.top_k(probs, k)
            tw = tw / jnp.sum(tw, axis=-1, keepdims=True)
        return (
            np.asarray(logits, np.float32),
            np.asarray(tw, np.float32),
            np.asarray(ti),
        )
    except Exception:
        logits = (x.astype(np.float64) @ gate_w.astype(np.float64).T).astype(
            np.float32
        )
        m = logits.max(-1, keepdims=True)
        p = np.exp(logits - m)
        p /= p.sum(-1, keepdims=True)
        ti = np.argsort(-p, axis=-1, kind="stable")[:, :k]
        tw = np.take_along_axis(p, ti, axis=-1)
        tw = tw / tw.sum(-1, keepdims=True)
        return logits, tw.astype(np.float32), ti


def _build(C):
    """One-expert SwiGLU MLP over C capacity-padded tokens (one core each).

    Inputs : xT [D, C] (tokens transposed), wg/wu [D, F], wd [F, D]
    Output : y [C, D] = silu(x @ wg) * (x @ wu) @ wd   (unweighted)
    """
    import concourse.mybir as mybir
    import concourse.tile as tile
    from concourse import bacc

    f32 = mybir.dt.float32
    f32r = mybir.dt.float32r
    Silu = mybir.ActivationFunctionType.Silu

    nc = bacc.Bacc()
    xT = nc.declare_dram_parameter("xT", [DK, P, C], f32r, isOutput=False)
    wg = nc.declare_dram_parameter("wg", [FK, P, DK, P], f32r, isOutput=False)
    wu = nc.declare_dram_parameter("wu", [FK, P, DK, P], f32r, isOutput=False)
    wd = nc.declare_dram_parameter(
        "wd", [D // NT, P, FK, NT], f32r, isOutput=False
    )
    y = nc.declare_dram_parameter("y", [C, D], f32, isOutput=True)

    tcs = _chunks(C)
    with tile.TileContext(nc) as tc:
        with (
            tc.tile_pool(name="xpool", bufs=1) as xpool,
            tc.tile_pool(name="wpool", bufs=4) as wpool,
            tc.tile_pool(name="hpool", bufs=1) as hpool,
            tc.tile_pool(name="wdpool", bufs=2) as wdpool,
            tc.tile_pool(name="ypool", bufs=4) as ypool,
            tc.tile_pool(name="psum", bufs=4, space="PSUM") as psum,
            tc.tile_pool(name="psumy", bufs=2, space="PSUM") as psumy,
        ):
            # Resident token activations, partition-major: xts[p, d, c].
            # One DMA per d-chunk (contiguous per-partition runs) so the
            # first matmuls start as soon as chunk 0 lands.
            xts = xpool.tile([P, DK, C], f32r)
            for n in range(DK):
                nc.sync.dma_start(xts[:, n, :], xT[n])

            # H^T = silu(x@wg)^T * (x@wu)^T, laid out [f_part, token]
            ht = hpool.tile([P, FK, C], f32r)

            for fi in range(FK):
                wgt = wpool.tile([P, DK, P], f32r, tag="w")
                wut = wpool.tile([P, DK, P], f32r, tag="w")
                nc.sync.dma_start(wgt[:], wg[fi])
                nc.sync.dma_start(wut[:], wu[fi])
                t0 = 0
                for tch in tcs:
                    tsl = slice(t0, t0 + tch)
                    t0 += tch
                    pg = psum.tile([P, tch], f32, tag="pg")
                    pu = psum.tile([P, tch], f32, tag="pg")
                    for d in range(DK):
                        nc.tensor.matmul(
                            pg[:],
                            wgt[:, d, :],
                            xts[:, d, tsl],
                            start=(d == 0),
                            stop=(d == DK - 1),
                        )
                    for d in range(DK):
                        nc.tensor.matmul(
                            pu[:],
                            wut[:, d, :],
                            xts[:, d, tsl],
                            start=(d == 0),
                            stop=(d == DK - 1),
                        )
                    nc.scalar.activation(ht[:, fi, tsl], pg[:], Silu)
                    nc.vector.tensor_tensor(
                        ht[:, fi, tsl],
                        ht[:, fi, tsl],
                        pu[:],
                        op=mybir.AluOpType.mult,
                    )

            # y = H @ wd, output tiled [token_part, dout]
            for n in range(D // NT):
                nsl = slice(n * NT, (n + 1) * NT)
                wdt = wdpool.tile([P, FK, NT], f32r)
                nc.sync.dma_start(wdt[:], wd[n])
                for t in range(C // P):
                    py = psumy.tile([P, NT], f32)
                    for f in range(FK):
                        nc.tensor.matmul(
                            py[:],
                            ht[:, f, t * P : (t + 1) * P],
                            wdt[:, f, :],
                            start=(f == 0),
                            stop=(f == FK - 1),
                        )
                    yt = ypool.tile([P, NT], f32)
                    nc.vector.tensor_copy(yt[:], py[:])
                    nc.sync.dma_start(y[t * P : (t + 1) * P, nsl], yt[:])
    return nc


def _get_nc(C):
    if C not in _COMPILED:
        nc = _build(C)
        nc.finalize()  # runs Bacc.compile(): wait-splitting + reg alloc
        _COMPILED[C] = nc
    return _COMPILED[C]


def kernel(hidden_states, gate_w, w_gate, w_up, w_down, top_k):
    global LAST_RESULT
    from concourse.bass_utils import run_bass_kernel_spmd

    x = np.ascontiguousarray(
        np.asarray(hidden_states, dtype=np.float32).reshape(T, D)
    )
    gate_w = np.asarray(gate_w, dtype=np.float32)
    k = int(top_k)

    logits, tw, ti = _route(x, gate_w, k)

    idxs, wts = [], []
    maxlen = 1
    for e in range(E):
        mask = ti == e  # [T, k]
        idx = np.nonzero(mask.any(axis=1))[0]
        w = (tw * mask).sum(axis=1)[idx]
        idxs.append(idx)
        wts.append(w.astype(np.float32))
        maxlen = max(maxlen, len(idx))
    C = ((maxlen + CQ - 1) // CQ) * CQ

    nc = _get_nc(C)
    xr = _round_fp32r(x)
    w_gate = np.asarray(w_gate, np.float32)
    w_up = np.asarray(w_up, np.float32)
    w_down = np.asarray(w_down, np.float32)

    def _pack_dxf(w):  # [D, F] -> [FK, 128, DK, 128] (partition-major)
        return np.ascontiguousarray(
            _round_fp32r(w).reshape(DK, P, FK, P).transpose(2, 1, 0, 3)
        )

    def _pack_fxd(w):  # [F, D] -> [D/NT, 128, FK, NT]
        return np.ascontiguousarray(
            _round_fp32r(w).reshape(FK, P, D // NT, NT).transpose(2, 1, 0, 3)
        )

    in_maps = []
    for e in range(E):
        xTe = np.zeros((DK, P, C), np.float32)
        n_e = len(idxs[e])
        xTe.reshape(D, C)[:, :n_e] = xr[idxs[e]].T
        in_maps.append(
            {
                "xT": xTe,
                "wg": _pack_dxf(w_gate[e]),
                "wu": _pack_dxf(w_up[e]),
                "wd": _pack_fxd(w_down[e]),
            }
        )

    trace = os.environ.get("BASS_MOE_TRACE", "0") == "1"
    LAST_RESULT = run_bass_kernel_spmd(
        nc, in_maps, list(range(E)), trace=trace
    )

    out = np.zeros((T, D), np.float32)
    for e in range(E):
        n_e = len(idxs[e])
        if n_e:
            out[idxs[e]] += wts[e][:, None] * LAST_RESULT.results[e]["y"][:n_e]
    return out.reshape(B, S, D), logits


# revision 13
# speedup vs baseline: 1.1848x; 1.1848x over previous
"""MiniMax MoE block on 8 trn2 NeuronCores.

Expert-parallel: one expert per core. The tiny router (T x D @ D x E) and the
top-k dispatch run on host as the sharding step; each core runs the SwiGLU
expert MLP over the tokens routed to it (capacity-padded), and the host
scatter-adds the weighted expert outputs back (the combine weight commutes
through the down projection, so it is applied during the combine).

Shapes (hardcoded): B=2, S=1024, D=2048, E=8, F=1024, top_k=2.
"""

import os

import numpy as np

B, S, D, E, F = 2, 1024, 2048, 8, 1024
T = B * S
P = 128
NT = 512  # max token chunk (matmul moving dim; fp32 max is 512)
CQ = 128  # capacity quantum; _chunks keeps every chunk >= 256
DK = D // P  # 16 contraction tiles over D
FK = F // P  # 8 contraction tiles over F


def _chunks(C):
    """Split C (multiple of 256) into balanced chunks of 256..512, each a
    multiple of 128, so every fp32r matmul keeps the full-rate moving dim."""
    k = (C + NT - 1) // NT
    base = (C // k) // P * P
    out = [base] * k
    rem = C - base * k
    i = 0
    while rem:
        out[i] += P
        rem -= P
        i += 1
    return out

_COMPILED = {}
LAST_RESULT = None


def _round_fp32r(a):
    """Round fp32 to the PE's fp32r format (11 mantissa bits, RNE)."""
    b = np.ascontiguousarray(a, np.float32).view(np.uint32)
    b = b + np.uint32(0x7FF) + ((b >> np.uint32(12)) & np.uint32(1))
    b &= np.uint32(0xFFFFF000)
    return b.view(np.float32)


def _route(x, gate_w, k):
    """Router logits + top-k combine weights, matching reference numerics.

    Uses jax on CPU with the exact op sequence of the reference so the top-k
    selection is bit-identical; falls back to numpy if jax is unavailable.
    """
    try:
        import jax
        import jax.numpy as jnp

        cpu = jax.devices("cpu")[0]
        with jax.default_device(cpu):
            xl = jnp.asarray(x)
            gw = jnp.asarray(gate_w)
            logits = jnp.einsum("td,ed->te", xl, gw)
            probs = jax.nn.softmax(logits, axis=-1)
            tw, ti = jax.lax.top_k(probs, k)
            tw = tw / jnp.sum(tw, axis=-1, keepdims=True)
        return (
            np.asarray(logits, np.float32),
            np.asarray(tw, np.float32),
            np.asarray(ti),
        )
    except Exception:
        logits = (x.astype(np.float64) @ gate_w.astype(np.float64).T).astype(
            np.float32
        )
        m = logits.max(-1, keepdims=True)
        p = np.exp(logits - m)
        p /= p.sum(-1, keepdims=True)
        ti = np.argsort(-p, axis=-1, kind="stable")[:, :k]
        tw = np.take_along_axis(p, ti, axis=-1)
        tw = tw / tw.sum(-1, keepdims=True)
        return logits, tw.astype(np.float32), ti


def _build(C):
    """One-expert SwiGLU MLP over C capacity-padded tokens (one core each).

    Inputs : xT [D, C] (tokens transposed), wg/wu [D, F], wd [F, D]
    Output : y [C, D] = silu(x @ wg) * (x @ wu) @ wd   (unweighted)
    """
    import concourse.mybir as mybir
    import concourse.tile as tile
    from concourse import bacc

    f32 = mybir.dt.float32
    f32r = mybir.dt.float32r
    Silu = mybir.ActivationFunctionType.Silu

    nc = bacc.Bacc()
    xT = nc.declare_dram_parameter("xT", [DK, P, C], f32r, isOutput=False)
    wg = nc.declare_dram_parameter("wg", [FK, P, DK, P], f32r, isOutput=False)
    wu = nc.declare_dram_parameter("wu", [FK, P, DK, P], f32r, isOutput=False)
    wd = nc.declare_dram_parameter(
        "wd", [D // NT, P, FK, NT], f32r, isOutput=False
    )
    y = nc.declare_dram_parameter("y", [C, D], f32, isOutput=True)

    tcs = _chunks(C)
    with tile.TileContext(nc) as tc:
        with (
            tc.tile_pool(name="xpool", bufs=1) as xpool,
            tc.tile_pool(name="wpool", bufs=4) as wpool,
            tc.tile_pool(name="hpool", bufs=1) as hpool,
            tc.tile_pool(name="wdpool", bufs=4) as wdpool,
            tc.tile_pool(name="ypool", bufs=4) as ypool,
            tc.tile_pool(name="psum", bufs=4, space="PSUM") as psum,
            tc.tile_pool(name="psumy", bufs=2, space="PSUM") as psumy,
        ):
            # Resident token activations, partition-major: xts[p, d, c].
            # One DMA per d-chunk (contiguous per-partition runs) so the
            # first matmuls start as soon as chunk 0 lands.
            xts = xpool.tile([P, DK, C], f32r)
            for n in range(DK):
                # ACT is also a HWDGE engine: dispatching x here runs in
                # parallel with the weight dispatches on the Sync queue.
                nc.scalar.dma_start(xts[:, n, :], xT[n])

            # H^T = silu(x@wg)^T * (x@wu)^T, laid out [f_part, token]
            ht = hpool.tile([P, FK, C], f32r)

            for fi in range(FK):
                wgt = wpool.tile([P, DK, P], f32r, tag="w")
                wut = wpool.tile([P, DK, P], f32r, tag="w")
                nc.sync.dma_start(wgt[:], wg[fi])
                nc.sync.dma_start(wut[:], wu[fi])
                t0 = 0
                for tch in tcs:
                    tsl = slice(t0, t0 + tch)
                    t0 += tch
                    pg = psum.tile([P, tch], f32, tag="pg")
                    pu = psum.tile([P, tch], f32, tag="pg")
                    for d in range(DK):
                        nc.tensor.matmul(
                            pg[:],
                            wgt[:, d, :],
                            xts[:, d, tsl],
                            start=(d == 0),
                            stop=(d == DK - 1),
                        )
                    for d in range(DK):
                        nc.tensor.matmul(
                            pu[:],
                            wut[:, d, :],
                            xts[:, d, tsl],
                            start=(d == 0),
                            stop=(d == DK - 1),
                        )
                    nc.scalar.activation(ht[:, fi, tsl], pg[:], Silu)
                    nc.vector.tensor_tensor(
                        ht[:, fi, tsl],
                        ht[:, fi, tsl],
                        pu[:],
                        op=mybir.AluOpType.mult,
                    )

            # Prefetch all of wd; emitted after the wg/wu dma_starts so
            # its descriptors queue behind them on the Sync FIFO and land
            # just before stage B needs them.
            wdts = []
            for n in range(D // NT):
                wdt = wdpool.tile([P, FK, NT], f32r)
                nc.sync.dma_start(wdt[:], wd[n])
                wdts.append(wdt)

            # y = H @ wd, output tiled [token_part, dout]
            for n in range(D // NT):
                nsl = slice(n * NT, (n + 1) * NT)
                wdt = wdts[n]
                for t in range(C // P):
                    py = psumy.tile([P, NT], f32)
                    for f in range(FK):
                        nc.tensor.matmul(
                            py[:],
                            ht[:, f, t * P : (t + 1) * P],
                            wdt[:, f, :],
                            start=(f == 0),
                            stop=(f == FK - 1),
                        )
                    yt = ypool.tile([P, NT], f32)
                    nc.vector.tensor_copy(yt[:], py[:])
                    nc.sync.dma_start(y[t * P : (t + 1) * P, nsl], yt[:])
    return nc


def _get_nc(C):
    if C not in _COMPILED:
        nc = _build(C)
        nc.finalize()  # runs Bacc.compile(): wait-splitting + reg alloc
        _COMPILED[C] = nc
    return _COMPILED[C]


def kernel(hidden_states, gate_w, w_gate, w_up, w_down, top_k):
    global LAST_RESULT
    from concourse.bass_utils import run_bass_kernel_spmd

    x = np.ascontiguousarray(
        np.asarray(hidden_states, dtype=np.float32).reshape(T, D)
    )
    gate_w = np.asarray(gate_w, dtype=np.float32)
    k = int(top_k)

    logits, tw, ti = _route(x, gate_w, k)

    idxs, wts = [], []
    maxlen = 1
    for e in range(E):
        mask = ti == e  # [T, k]
        idx = np.nonzero(mask.any(axis=1))[0]
        w = (tw * mask).sum(axis=1)[idx]
        idxs.append(idx)
        wts.append(w.astype(np.float32))
        maxlen = max(maxlen, len(idx))
    C = ((maxlen + CQ - 1) // CQ) * CQ

    nc = _get_nc(C)
    xr = _round_fp32r(x)
    w_gate = np.asarray(w_gate, np.float32)
    w_up = np.asarray(w_up, np.float32)
    w_down = np.asarray(w_down, np.float32)

    def _pack_dxf(w):  # [D, F] -> [FK, 128, DK, 128] (partition-major)
        return np.ascontiguousarray(
            _round_fp32r(w).reshape(DK, P, FK, P).transpose(2, 1, 0, 3)
        )

    def _pack_fxd(w):  # [F, D] -> [D/NT, 128, FK, NT]
        return np.ascontiguousarray(
            _round_fp32r(w).reshape(FK, P, D // NT, NT).transpose(2, 1, 0, 3)
        )

    in_maps = []
    for e in range(E):
        xTe = np.zeros((DK, P, C), np.float32)
        n_e = len(idxs[e])
        xTe.reshape(D, C)[:, :n_e] = xr[idxs[e]].T
        in_maps.append(
            {
                "xT": xTe,
                "wg": _pack_dxf(w_gate[e]),
                "wu": _pack_dxf(w_up[e]),
                "wd": _pack_fxd(w_down[e]),
            }
        )

    trace = os.environ.get("BASS_MOE_TRACE", "0") == "1"
    LAST_RESULT = run_bass_kernel_spmd(
        nc, in_maps, list(range(E)), trace=trace
    )

    out = np.zeros((T, D), np.float32)
    for e in range(E):
        n_e = len(idxs[e])
        if n_e:
            out[idxs[e]] += wts[e][:, None] * LAST_RESULT.results[e]["y"][:n_e]
    return out.reshape(B, S, D), logits


# revision 15
# speedup vs baseline: 1.2400x; 1.0466x over previous
"""MiniMax MoE block on 8 trn2 NeuronCores.

Expert-parallel: one expert per core. The tiny router (T x D @ D x E) and the
top-k dispatch run on host as the sharding step; each core runs the SwiGLU
expert MLP over the tokens routed to it (capacity-padded), and the host
scatter-adds the weighted expert outputs back (the combine weight commutes
through the down projection, so it is applied during the combine).

Shapes (hardcoded): B=2, S=1024, D=2048, E=8, F=1024, top_k=2.
"""

import os

import numpy as np

B, S, D, E, F = 2, 1024, 2048, 8, 1024
T = B * S
P = 128
NT = 512  # max token chunk (matmul moving dim; fp32 max is 512)
CQ = 128  # capacity quantum; _chunks keeps every chunk >= 256
DK = D // P  # 16 contraction tiles over D
FK = F // P  # 8 contraction tiles over F


def _chunks(C):
    """Split C (multiple of 256) into balanced chunks of 256..512, each a
    multiple of 128, so every fp32r matmul keeps the full-rate moving dim."""
    k = (C + NT - 1) // NT
    base = (C // k) // P * P
    out = [base] * k
    rem = C - base * k
    i = 0
    while rem:
        out[i] += P
        rem -= P
        i += 1
    return out

_COMPILED = {}
LAST_RESULT = None


def _round_fp32r(a):
    """Round fp32 to the PE's fp32r format (11 mantissa bits, RNE)."""
    b = np.ascontiguousarray(a, np.float32).view(np.uint32)
    b = b + np.uint32(0x7FF) + ((b >> np.uint32(12)) & np.uint32(1))
    b &= np.uint32(0xFFFFF000)
    return b.view(np.float32)


def _route(x, gate_w, k):
    """Router logits + top-k combine weights, matching reference numerics.

    Uses jax on CPU with the exact op sequence of the reference so the top-k
    selection is bit-identical; falls back to numpy if jax is unavailable.
    """
    try:
        import jax
        import jax.numpy as jnp

        cpu = jax.devices("cpu")[0]
        with jax.default_device(cpu):
            xl = jnp.asarray(x)
            gw = jnp.asarray(gate_w)
            logits = jnp.einsum("td,ed->te", xl, gw)
            probs = jax.nn.softmax(logits, axis=-1)
            tw, ti = jax.lax.top_k(probs, k)
            tw = tw / jnp.sum(tw, axis=-1, keepdims=True)
        return (
            np.asarray(logits, np.float32),
            np.asarray(tw, np.float32),
            np.asarray(ti),
        )
    except Exception:
        logits = (x.astype(np.float64) @ gate_w.astype(np.float64).T).astype(
            np.float32
        )
        m = logits.max(-1, keepdims=True)
        p = np.exp(logits - m)
        p /= p.sum(-1, keepdims=True)
        ti = np.argsort(-p, axis=-1, kind="stable")[:, :k]
        tw = np.take_along_axis(p, ti, axis=-1)
        tw = tw / tw.sum(-1, keepdims=True)
        return logits, tw.astype(np.float32), ti


def _build(C):
    """One-expert SwiGLU MLP over C capacity-padded tokens (one core each).

    Inputs : xT [D, C] (tokens transposed), wg/wu [D, F], wd [F, D]
    Output : y [C, D] = silu(x @ wg) * (x @ wu) @ wd   (unweighted)
    """
    import concourse.mybir as mybir
    import concourse.tile as tile
    from concourse import bacc

    f32 = mybir.dt.float32
    f32r = mybir.dt.float32r
    Silu = mybir.ActivationFunctionType.Silu

    nc = bacc.Bacc()
    xT = nc.declare_dram_parameter("xT", [DK, P, C], f32r, isOutput=False)
    wg = nc.declare_dram_parameter("wg", [FK, P, DK, P], f32r, isOutput=False)
    wu = nc.declare_dram_parameter("wu", [FK, P, DK, P], f32r, isOutput=False)
    wd = nc.declare_dram_parameter(
        "wd", [D // NT, P, FK, NT], f32r, isOutput=False
    )
    y = nc.declare_dram_parameter("y", [C, D], f32, isOutput=True)

    tcs = _chunks(C)
    with tile.TileContext(nc) as tc:
        with (
            tc.tile_pool(name="xpool", bufs=1) as xpool,
            tc.tile_pool(name="wpool", bufs=4) as wpool,
            tc.tile_pool(name="hpool", bufs=1) as hpool,
            tc.tile_pool(name="wdpool", bufs=4) as wdpool,
            tc.tile_pool(name="ypool", bufs=2) as ypool,
            tc.tile_pool(name="psum", bufs=4, space="PSUM") as psum,
            tc.tile_pool(name="psumy", bufs=4, space="PSUM") as psumy,
        ):
            # Resident token activations, partition-major: xts[p, d, c].
            # One DMA per d-chunk (contiguous per-partition runs) so the
            # first matmuls start as soon as chunk 0 lands.
            xts = xpool.tile([P, DK, C], f32r)

            # H^T = silu(x@wg)^T * (x@wu)^T, laid out [f_part, token]
            ht = hpool.tile([P, FK, C], f32r)

            for fi in range(FK):
                wgt = wpool.tile([P, DK, P], f32r, tag="w")
                wut = wpool.tile([P, DK, P], f32r, tag="w")
                nc.sync.dma_start(wgt[:], wg[fi])
                nc.sync.dma_start(wut[:], wu[fi])
                if fi == 0:
                    # x chunks dispatched right after the first weight pair
                    # so the fi=0 matmuls can start as soon as chunk 0 lands.
                    for n in range(DK):
                        nc.sync.dma_start(xts[:, n, :], xT[n])
                t0 = 0
                for tch in tcs:
                    tsl = slice(t0, t0 + tch)
                    t0 += tch
                    pg = psum.tile([P, tch], f32, tag="pg")
                    pu = psum.tile([P, tch], f32, tag="pg")
                    for d in range(DK):
                        nc.tensor.matmul(
                            pg[:],
                            wgt[:, d, :],
                            xts[:, d, tsl],
                            start=(d == 0),
                            stop=(d == DK - 1),
                        )
                    for d in range(DK):
                        nc.tensor.matmul(
                            pu[:],
                            wut[:, d, :],
                            xts[:, d, tsl],
                            start=(d == 0),
                            stop=(d == DK - 1),
                        )
                    nc.scalar.activation(ht[:, fi, tsl], pg[:], Silu)
                    nc.vector.tensor_tensor(
                        ht[:, fi, tsl],
                        ht[:, fi, tsl],
                        pu[:],
                        op=mybir.AluOpType.mult,
                    )

            # Prefetch all of wd; emitted after the wg/wu dma_starts so
            # its descriptors queue behind them on the Sync FIFO and land
            # just before stage B needs them.
            wdts = []
            for n in range(D // NT):
                wdt = wdpool.tile([P, FK, NT], f32r)
                nc.sync.dma_start(wdt[:], wd[n])
                wdts.append(wdt)

            # y = H @ wd: t outer, all 4 dout chunks accumulated together
            # (4 PSUM banks); MM3 groups reuse the stationary ht tile and y
            # goes out as one contiguous [128, D] DMA per token tile.
            ND = D // NT
            for t in range(C // P):
                pys = [
                    psumy.tile([P, NT], f32, tag="py", name=f"py{i}")
                    for i in range(ND)
                ]
                for f in range(FK):
                    for n in range(ND):
                        nc.tensor.matmul(
                            pys[n][:],
                            ht[:, f, t * P : (t + 1) * P],
                            wdts[n][:, f, :],
                            start=(f == 0),
                            stop=(f == FK - 1),
                        )
                yt = ypool.tile([P, D], f32)
                for n in range(ND):
                    nc.vector.tensor_copy(yt[:, n * NT : (n + 1) * NT], pys[n][:])
                nc.sync.dma_start(y[t * P : (t + 1) * P, :], yt[:])
    return nc


def _get_nc(C):
    if C not in _COMPILED:
        nc = _build(C)
        nc.finalize()  # runs Bacc.compile(): wait-splitting + reg alloc
        _COMPILED[C] = nc
    return _COMPILED[C]


def kernel(hidden_states, gate_w, w_gate, w_up, w_down, top_k):
    global LAST_RESULT
    from concourse.bass_utils import run_bass_kernel_spmd

    x = np.ascontiguousarray(
        np.asarray(hidden_states, dtype=np.float32).reshape(T, D)
    )
    gate_w = np.asarray(gate_w, dtype=np.float32)
    k = int(top_k)

    logits, tw, ti = _route(x, gate_w, k)

    idxs, wts = [], []
    maxlen = 1
    for e in range(E):
        mask = ti == e  # [T, k]
        idx = np.nonzero(mask.any(axis=1))[0]
        w = (tw * mask).sum(axis=1)[idx]
        idxs.append(idx)
        wts.append(w.astype(np.float32))
        maxlen = max(maxlen, len(idx))
    C = ((maxlen + CQ - 1) // CQ) * CQ

    nc = _get_nc(C)
    xr = _round_fp32r(x)
    w_gate = np.asarray(w_gate, np.float32)
    w_up = np.asarray(w_up, np.float32)
    w_down = np.asarray(w_down, np.float32)

    def _pack_dxf(w):  # [D, F] -> [FK, 128, DK, 128] (partition-major)
        return np.ascontiguousarray(
            _round_fp32r(w).reshape(DK, P, FK, P).transpose(2, 1, 0, 3)
        )

    def _pack_fxd(w):  # [F, D] -> [D/NT, 128, FK, NT]
        return np.ascontiguousarray(
            _round_fp32r(w).reshape(FK, P, D // NT, NT).transpose(2, 1, 0, 3)
        )

    in_maps = []
    for e in range(E):
        xTe = np.zeros((DK, P, C), np.float32)
        n_e = len(idxs[e])
        xTe.reshape(D, C)[:, :n_e] = xr[idxs[e]].T
        in_maps.append(
            {
                "xT": xTe,
                "wg": _pack_dxf(w_gate[e]),
                "wu": _pack_dxf(w_up[e]),
                "wd": _pack_fxd(w_down[e]),
            }
        )

    trace = os.environ.get("BASS_MOE_TRACE", "0") == "1"
    LAST_RESULT = run_bass_kernel_spmd(
        nc, in_maps, list(range(E)), trace=trace
    )

    out = np.zeros((T, D), np.float32)
    for e in range(E):
        n_e = len(idxs[e])
        if n_e:
            out[idxs[e]] += wts[e][:, None] * LAST_RESULT.results[e]["y"][:n_e]
    return out.reshape(B, S, D), logits
